# revision 5
# baseline (speedup 1.0000x reference)
"""Trainium2 Bass kernel for nn_ConstraintAwareBiasing.

Computes bias[b, n, i, j] = temp[n] * (relu(relu(hi[b,i] + hj[b,j]) @ W2 + b2) @ W3 + b3)[n]
with hi = x @ W1[:128] + b1, hj = x @ W1[128:], masked by `mask`.

Strategy (8 NeuronCores):
  - Shard the (b, i) query axis: core = b*4 + chunk, each core owns 128 i-rows
    against all 512 j for one batch element.
  - Host precomputes hi/hj (tiny [512,128] matmuls), folds head_temperatures
    into W3, adds b3*temp and applies the mask on the host.
  - On device, per query row i:
      s1: h1 = relu(hjT + hi_col)    DVE tensor_scalar, all operands bf16 so
                                     the 4x_2p perf mode engages
      W2: p1 = W2^T @ h1             PE matmul -> PSUM (pairs of i share a
                                     2-bank PSUM tile)
      s3: h2 = relu(p1 + b2)         ACT/DVE PSUM->SBUF pass (split to
                                     balance engine occupancy)
      W3: p2[32c+16f : +16] += W3f^T @ h2
                                     PE matmul, col-tiled 4x; two phases f
                                     with zero-padded M=32 stationaries
                                     ([W3|0] then accumulate [0|W3]) pack
                                     EIGHT i-rows into one PSUM bank
      s5: ot = copy(p2) bf16         one PSUM->SBUF pass per 8 rows
      DMA ships each ot slab (bf16, halves output bytes); host up-casts,
      reorders, and adds b3/mask.
    Stage 2 of group g-1 is emitted interleaved with stage 1 of group g
    (software pipelining) so the in-order engine streams don't block.
"""

import numpy as np
import ml_dtypes

import concourse.bass as bass
import concourse.tile as tile
import concourse.mybir as mybir
from concourse import bacc
from concourse.bass_utils import run_bass_kernel_spmd

BF16 = ml_dtypes.bfloat16

B, S, D = 2, 512, 128          # batch, seq, state dim
H, NH = 128, 16                # hidden, heads
N_CORES = 8
CHUNKS = N_CORES // B          # i-chunks per batch element
I_PER_CORE = S // CHUNKS       # 128
GROUPS = I_PER_CORE // 4       # 4 i-rows per group; 2 groups share a p2 bank
NEG_INF = float("-inf")

_CACHE: dict = {}

# Engine-assignment patterns (tuned against NTFF profiles).
# s1 per i (i % len): "v" = VectorE, "g" = GpSimdE
S1_PAT = ["v"] * 8
# s3 per pair index (pi % len): "a" = ScalarE, "v" = VectorE.
# DVE also carries s1 + s5, so ACT takes ~3/4 of s3.
S3_PAT = ["a", "a", "a", "v"] * 4
# s5 per 8-i slab (sl % len)
S5_PAT = ["v", "v"]


def _build_bass():
    nc = bacc.Bacc("TRN2")
    dt = mybir.dt
    hj_d = nc.dram_tensor("hj", (H, S), dt.bfloat16, kind="ExternalInput")
    hi_d = nc.dram_tensor("hi", (H, I_PER_CORE), dt.float32, kind="ExternalInput")
    w2_d = nc.dram_tensor("w2", (H, H), dt.bfloat16, kind="ExternalInput")
    # Two zero-padded W3 stationaries: w3a = [W3 | 0], w3b = [0 | W3], each
    # [H, 32]. Phase a writes rows 32c..32c+15 of the bank, phase b
    # accumulates rows 32c+16..32c+31 (its top 16 rows add zero).
    w3a_d = nc.dram_tensor("w3a", (H, 32), dt.bfloat16, kind="ExternalInput")
    w3b_d = nc.dram_tensor("w3b", (H, 32), dt.bfloat16, kind="ExternalInput")
    b2_d = nc.dram_tensor("b2", (H, 1), dt.float32, kind="ExternalInput")
    out_d = nc.dram_tensor("out", (GROUPS // 2, H, S), dt.bfloat16,
                           kind="ExternalOutput")

    relu = mybir.ActivationFunctionType.Relu
    add, amax = mybir.AluOpType.add, mybir.AluOpType.max

    with tile.TileContext(nc) as tc:
        with tc.tile_pool(name="singles", bufs=1) as singles, \
             tc.tile_pool(name="h1p", bufs=10) as h1p, \
             tc.tile_pool(name="h2p", bufs=8) as h2p, \
             tc.tile_pool(name="otp", bufs=3) as otp, \
             tc.tile_pool(name="ps1", bufs=3, space="PSUM") as ps1, \
             tc.tile_pool(name="ps2", bufs=2, space="PSUM") as ps2:
            hj = singles.tile([H, S], dt.bfloat16)
            hi = singles.tile([H, I_PER_CORE], dt.float32)
            w2 = singles.tile([H, H], dt.bfloat16)
            w3a = singles.tile([H, 32], dt.bfloat16)
            w3b = singles.tile([H, 32], dt.bfloat16)
            b2 = singles.tile([H, 1], dt.float32)
            # dummy relu first: pulls the ~2.7us ACT table load into the
            # input-DMA wait window instead of serializing at the first s3
            warm = singles.tile([128, 1], dt.float32)
            nc.vector.memset(warm[:], 0.0)
            nc.scalar.activation(out=warm[:], in_=warm[:], func=relu)
            nc.sync.dma_start(out=hj[:], in_=hj_d[:])
            nc.scalar.dma_start(out=hi[:], in_=hi_d[:])
            for t, d in [(w2, w2_d), (w3a, w3a_d), (w3b, w3b_d), (b2, b2_d)]:
                nc.sync.dma_start(out=t[:], in_=d[:])

            # 1-group software pipeline: stage2 (W3 matmuls, s5, DMA) of
            # group g-1 is emitted interleaved with stage1 (s1, W2, s3) of
            # group g so in-order engine streams never head-of-line block.
            pend = None   # (g, h2_pair_tiles) awaiting stage2

            p2_state = {"tile": None}

            def stage2_w3(g, h2g):
                phase = g % 2
                if phase == 0:
                    p2_state["tile"] = ps2.tile([128, S], dt.float32,
                                                name="p2", tag="p2")
                p2 = p2_state["tile"]
                w3f = w3a if phase == 0 else w3b
                for c in range(4):
                    nc.tensor.matmul(
                        p2[32 * c:32 * c + 32, :], lhsT=w3f[:],
                        rhs=h2g[c // 2][:, (c % 2) * S:(c % 2 + 1) * S],
                        start=(phase == 0), stop=(phase == 1),
                        tile_position=(0, 32 * c))
                return p2

            def stage2_out(g, p2):
                if g % 2 != 1:
                    return
                sl = (g - 1) // 2
                ot = otp.tile([128, S], dt.bfloat16, name="ot", tag="ot")
                if S5_PAT[sl % len(S5_PAT)] == "v":
                    nc.vector.tensor_copy(ot[:], p2[:])
                else:
                    nc.scalar.copy(out=ot[:], in_=p2[:])
                nc.sync.dma_start(out=out_d[sl], in_=ot[:])

            for g in range(GROUPS + 1):
                if pend is not None:
                    p2 = stage2_w3(*pend)   # PE: inputs ready since last iter

                if g < GROUPS:
                    q = [ps1.tile([H, 2 * S], dt.float32, name=f"q{_p}", tag="q") for _p in range(2)]
                    h2 = [h2p.tile([H, 2 * S], dt.bfloat16, name=f"h2_{_p}", tag="h2") for _p in range(2)]
                    for p in range(2):
                        for c in (2 * p, 2 * p + 1):
                            i = 4 * g + c
                            h1 = h1p.tile([H, S], dt.bfloat16)
                            s1_eng = {"v": nc.vector,
                                      "g": nc.gpsimd}[S1_PAT[i % len(S1_PAT)]]
                            s1_eng.tensor_scalar(
                                out=h1[:], in0=hj[:], scalar1=hi[:, i:i + 1],
                                scalar2=0.0, op0=add, op1=amax)
                            nc.tensor.matmul(
                                q[p][:, (c % 2) * S:(c % 2 + 1) * S],
                                lhsT=w2[:], rhs=h1[:], start=True, stop=True)
                        pi = 2 * g + p   # global pair index
                        if S3_PAT[pi % len(S3_PAT)] == "a":
                            nc.scalar.activation(out=h2[p][:], in_=q[p][:],
                                                 func=relu, bias=b2[:], scale=1.0)
                        else:
                            nc.vector.tensor_scalar(
                                out=h2[p][:], in0=q[p][:], scalar1=b2[:, 0:1],
                                scalar2=0.0, op0=add, op1=amax)

                if pend is not None:
                    stage2_out(pend[0], p2)
                pend = (g, h2) if g < GROUPS else None
    nc.compile()
    return nc


def _host_prep(inputs):
    x = np.asarray(inputs["state_embeddings"], dtype=np.float32)   # [B, S, D]
    W1 = np.asarray(inputs["W1"], dtype=np.float32)                # [2D, H]
    b1 = np.asarray(inputs["b1"], dtype=np.float32)                # [H]
    W2 = np.asarray(inputs["W2"], dtype=np.float32)                # [H, H]
    b2 = np.asarray(inputs["b2"], dtype=np.float32)                # [H]
    W3 = np.asarray(inputs["W3"], dtype=np.float32)                # [H, NH]
    b3 = np.asarray(inputs["b3"], dtype=np.float32)                # [NH]
    temp = np.asarray(inputs["head_temperatures"], dtype=np.float32)  # [NH]

    hi = x @ W1[:D] + b1                                           # [B, S, H]
    hj = x @ W1[D:]                                                # [B, S, H]
    w3p = (W3 * temp[None, :]).astype(BF16)                        # temp folded in
    b3p = b3 * temp                                                # added on host

    w3pad = np.zeros((H, 32), dtype=BF16)
    w3pad[:, :NH] = w3p
    w3a = np.ascontiguousarray(w3pad)
    w3b = np.ascontiguousarray(np.roll(w3pad, NH, axis=1))

    b2col = np.ascontiguousarray(b2.reshape(H, 1))

    in_maps = []
    for core in range(N_CORES):
        b, chunk = divmod(core, CHUNKS)
        i0 = chunk * I_PER_CORE
        in_maps.append({
            "hj": np.ascontiguousarray(hj[b].T).astype(BF16),                  # [H, S]
            "hi": np.ascontiguousarray(hi[b, i0:i0 + I_PER_CORE].T,
                                       dtype=np.float32),                      # [H, I]
            "w2": W2.astype(BF16),
            "w3a": w3a,
            "w3b": w3b,
            "b2": b2col,
        })
    return in_maps, b3p


def _assemble(results, inputs, b3p):
    mask = np.asarray(inputs["mask"])
    out = np.empty((B, NH, S, S), dtype=np.float32)
    for core in range(N_CORES):
        b, chunk = divmod(core, CHUNKS)
        i0 = chunk * I_PER_CORE
        # core result: [slab, 128, S] bf16; partition 32c+16f+n holds
        # (i = 8*slab + 4f + c, head n)
        r = results[core]["out"].astype(np.float32)
        r = r.reshape(GROUPS // 2, 4, 2, NH, S)        # [sl, c, f, n, j]
        r = r.transpose(3, 0, 2, 1, 4)                 # [n, sl, f, c, j]
        out[b, :, i0:i0 + I_PER_CORE, :] = r.reshape(NH, I_PER_CORE, S)
    if b3p.any():
        out += b3p[None, :, None, None]
    if not mask.all():
        out = np.where(mask[:, None, :, :], out, np.float32(NEG_INF))
    return out


def _get_nc():
    if "nc" not in _CACHE:
        _CACHE["nc"] = _build_bass()
    return _CACHE["nc"]


def run(inputs, trace=False):
    nc = _get_nc()
    in_maps, b3p = _host_prep(inputs)
    res = run_bass_kernel_spmd(nc, in_maps, core_ids=list(range(N_CORES)),
                               trace=trace)
    out = _assemble(res.results, inputs, b3p)
    return out, res


def kernel(**inputs) -> np.ndarray:
    out, _ = run(inputs, trace=False)
    return out
